# revision 1
# baseline (speedup 1.0000x reference)
"""PhiHarmonicAttention (B=1, S=2048, D=2048, H=16, Dh=128) on 8 Trainium2 cores.

Sharding: tensor-parallel over heads - 2 heads per core.
  - Wq/Wk/Wv column-sliced (256 cols per core), Wo row-sliced (256 rows).
  - Host sums the 8 partial outputs (TP row-parallel reduction).

v3: fp16 operands everywhere (full PE rate, fp32 PSUM accumulate, ~5e-4
rounding vs the 2e-2 budget; fp16 also unlocks the DVE 16-bit 2x mode used in
the rope path).  Exact causal clipping.  Softmax denominator via per-block
ones-matmuls accumulated in a dedicated PSUM bank.  Software-pipelined
emission: per 512-query chunk ci, PE work for B(ci) attention interleaves with
A(ci+1) projections and C(ci-1) output-projection units, and the chunk-0
projection pass is rotated to the loop tail so the hardware rep-loop wraps
without a PE stall.  PSUM static plan: 3 banks qkv accumulators, 3 banks
scores/out-proj rotation, 1 bank attention-output accumulator, 1 bank softmax
denominator.
"""
import numpy as np
from contextlib import ExitStack, nullcontext

import concourse.bass as bass
import concourse.tile as tile
from concourse import bacc, mybir
from concourse.bass_utils import run_bass_kernel_spmd

S = 2048
D = 2048
H = 16
DH = 128
NCORES = 8
HPC = H // NCORES          # heads per core = 2
CW = HPC * DH              # weight col-slice per core = 256
NO = D // 128              # contraction chunks = 16
NJ = S // 512              # rope table chunks = 4
NB = S // 128              # 128-wide seq blocks = 16
SCALE = float(1.0 / np.sqrt(np.float32(DH)))

ROT_FACTOR = (1.0 + 5.0 ** 0.5) / 2.0 - 1.0
ROPE_BASE = 10000.0

F32 = mybir.dt.float32
F32R = mybir.dt.float32r
F16 = mybir.dt.float16


def _build_nc(reps=1, stages="ABC"):
    nc = bacc.Bacc("TRN2", target_bir_lowering=False, debug=False, num_devices=NCORES)

    xt_d = nc.dram_tensor("xt", [D, S], F16, kind="ExternalInput").ap()
    wq_d = nc.dram_tensor("wq", [D, CW], F16, kind="ExternalInput").ap()
    wk_d = nc.dram_tensor("wk", [D, CW], F16, kind="ExternalInput").ap()
    wv_d = nc.dram_tensor("wv", [D, CW], F16, kind="ExternalInput").ap()
    wo_d = nc.dram_tensor("wo", [CW, D], F16, kind="ExternalInput").ap()
    rcu_d = nc.dram_tensor("ropecu", [DH, 512], F32, kind="ExternalInput").ap()
    rsu_d = nc.dram_tensor("ropesu", [DH, 512], F32, kind="ExternalInput").ap()
    rc512_d = nc.dram_tensor("ropec512", [DH, NJ], F32, kind="ExternalInput").ap()
    rs512_d = nc.dram_tensor("ropes512", [DH, NJ], F32, kind="ExternalInput").ap()
    onc_d = nc.dram_tensor("onescol", [128, 1], F16, kind="ExternalInput").ap()
    out_d = nc.dram_tensor("out", [S, D], F16, kind="ExternalOutput").ap()

    with ExitStack() as ctx:
        tc = ctx.enter_context(tile.TileContext(nc))
        consts = ctx.enter_context(tc.tile_pool(name="consts", bufs=1))
        persist = ctx.enter_context(tc.tile_pool(name="persist", bufs=1))
        xw = ctx.enter_context(tc.tile_pool(name="xw", bufs=28))
        ptp = ctx.enter_context(tc.tile_pool(name="ptp", bufs=8))
        work = ctx.enter_context(tc.tile_pool(name="work", bufs=3))
        recp = ctx.enter_context(tc.tile_pool(name="recp", bufs=3))
        outp = ctx.enter_context(tc.tile_pool(name="outp", bufs=4))
        psQ = ctx.enter_context(tc.tile_pool(name="psQ", bufs=3, space="PSUM"))
        psB = ctx.enter_context(tc.tile_pool(name="psB", bufs=3, space="PSUM"))
        psO = ctx.enter_context(tc.tile_pool(name="psO", bufs=1, space="PSUM"))
        psD = ctx.enter_context(tc.tile_pool(name="psD", bufs=1, space="PSUM"))

        # ---- constants ----
        wq_s = consts.tile([128, NO, CW], F16, tag="wq")
        wk_s = consts.tile([128, NO, CW], F16, tag="wk")
        wv_s = consts.tile([128, NO, CW], F16, tag="wv")
        wo_s = consts.tile([128, HPC, D], F16, tag="wo")
        rc = consts.tile([DH, S], F16, tag="rc")
        rs = consts.tile([DH, S], F16, tag="rs")
        tri = consts.tile([128, 128], F16, tag="tri")
        onc = consts.tile([128, 1], F16, tag="onc")
        rcu = consts.tile([DH, 512], F32, tag="rcu")
        rsu = consts.tile([DH, 512], F32, tag="rsu")
        rc512 = consts.tile([DH, NJ], F32, tag="rc512")
        rs512 = consts.tile([DH, NJ], F32, tag="rs512")

        nc.sync.dma_start(rcu[:], rcu_d)
        nc.sync.dma_start(rsu[:], rsu_d)
        nc.sync.dma_start(rc512[:], rc512_d)
        nc.sync.dma_start(rs512[:], rs512_d)
        nc.sync.dma_start(wq_s[:], wq_d.rearrange("(o p) n -> p o n", p=128))
        nc.sync.dma_start(wv_s[:], wv_d.rearrange("(o p) n -> p o n", p=128))
        nc.sync.dma_start(wk_s[:], wk_d.rearrange("(o p) n -> p o n", p=128))
        nc.sync.dma_start(onc[:], onc_d)
        nc.sync.dma_start(wo_s[:], wo_d.rearrange("(h p) n -> p h n", p=128))

        # rc/rs = full [128, 2048] rope tables via angle addition from the
        # 512-wide unit tables.
        for j in range(NJ):
            sl = slice(512 * j, 512 * (j + 1))
            tm = work.tile([128, 512], F32, tag="tm", name=f"tm{j}")
            nc.vector.tensor_scalar_mul(tm[:], rsu[:], rs512[:, j:j + 1])
            nc.vector.scalar_tensor_tensor(
                rc[:, sl], rcu[:], rc512[:, j:j + 1], tm[:],
                mybir.AluOpType.mult, mybir.AluOpType.subtract,
            )
            tm2 = work.tile([128, 512], F32, tag="tm2", name=f"tm2{j}")
            nc.vector.tensor_scalar_mul(tm2[:], rcu[:], rs512[:, j:j + 1])
            nc.vector.scalar_tensor_tensor(
                rs[:, sl], rsu[:], rc512[:, j:j + 1], tm2[:],
                mybir.AluOpType.mult, mybir.AluOpType.add,
            )
        # tri[p, c] = 1 if c >= p else 0  (within-block causal triangle)
        iot = work.tile([128, 512], F32, tag="tm", name="iot")
        nc.gpsimd.iota(
            iot[:, :128], pattern=[[1, 128]], base=0, channel_multiplier=-1,
            allow_small_or_imprecise_dtypes=True,
        )
        nc.vector.tensor_scalar(
            tri[:], iot[:, :128], 0.0, None, mybir.AluOpType.is_ge,
        )

        # ---- persistent per-head tensors ----
        qT = [persist.tile([DH, S], F16, tag=f"qT{h}", name=f"qT{h}")
              for h in range(HPC)]
        kT = [persist.tile([DH, S], F16, tag=f"kT{h}", name=f"kT{h}")
              for h in range(HPC)]
        v_sb = persist.tile([128, NB, CW], F16, tag="v")
        aT = [persist.tile([DH, S], F16, tag=f"aT{h}", name=f"aT{h}")
              for h in range(HPC)]

        xt_tiles = {}
        pqd, pkd, pvd = {}, {}, {}

        def xt_dma(ci):
            s0 = 512 * ci
            for o2 in range(NO // 2):
                t = xw.tile([128, 2, 512], F16, tag="xt", name=f"xt{ci}_{o2}")
                nc.sync.dma_start(
                    t[:],
                    xt_d[256 * o2:256 * (o2 + 1), s0:s0 + 512].rearrange(
                        "(a p) n -> p a n", p=128
                    ),
                )
                xt_tiles[(ci, 2 * o2)] = t[:, 0, :]
                xt_tiles[(ci, 2 * o2 + 1)] = t[:, 1, :]

        def rope_emit(psrc, dstT, s0):
            # dstT = raw*cos + halfswap(raw)*sin  (sign folded into rs)
            sl = slice(s0, s0 + 512)
            raw = work.tile([128, 512], F16, tag="raw")
            nc.scalar.copy(raw[:], psrc[:])          # frees the PSUM bank
            rsw = work.tile([128, 512], F16, tag="rsw")
            nc.gpsimd.tensor_copy(rsw[0:64, :], raw[64:128, :])
            nc.gpsimd.tensor_copy(rsw[64:128, :], raw[0:64, :])
            t1 = work.tile([128, 512], F16, tag="t1")
            nc.vector.tensor_mul(t1[:], raw[:], rc[:, sl])
            nc.vector.tensor_mul(rsw[:], rsw[:], rs[:, sl])
            nc.vector.tensor_add(dstT[:, sl], t1[:], rsw[:])

        def a_q_unit(ci, o):
            def f():
                if o == 0:
                    for h in range(HPC):
                        pqd[(ci, h)] = psQ.tile(
                            [128, 512], F32, tag="ps", name=f"pq{ci}_{h}"
                        )
                st = dict(start=(o == 0), stop=(o == NO - 1))
                xt_t = xt_tiles[(ci, o)]
                for h in range(HPC):
                    nc.tensor.matmul(
                        pqd[(ci, h)][:], wq_s[:, o, 128 * h:128 * (h + 1)],
                        xt_t, **st
                    )
                if o == NO - 1:
                    for h in range(HPC):
                        rope_emit(pqd[(ci, h)], qT[h], 512 * ci)
            return f

        def a_k_unit(ci, o):
            def f():
                if o == 0:
                    for h in range(HPC):
                        pkd[(ci, h)] = psQ.tile(
                            [128, 512], F32, tag="ps", name=f"pk{ci}_{h}"
                        )
                st = dict(start=(o == 0), stop=(o == NO - 1))
                xt_t = xt_tiles[(ci, o)]
                for h in range(HPC):
                    nc.tensor.matmul(
                        pkd[(ci, h)][:], wk_s[:, o, 128 * h:128 * (h + 1)],
                        xt_t, **st
                    )
                if o == NO - 1:
                    for h in range(HPC):
                        rope_emit(pkd[(ci, h)], kT[h], 512 * ci)
            return f

        def a_v_unit(ci, o, half):
            # v projection in two sequential half-passes (m4 in {0,1} then
            # {2,3}) so only 2 of the 3 psQ banks are held at a time.
            def f():
                if o == 0:
                    for i in range(2):
                        pvd[(ci, half, i)] = psQ.tile(
                            [128, 256], F32, tag="ps",
                            name=f"pv{ci}_{half}_{i}"
                        )
                st = dict(start=(o == 0), stop=(o == NO - 1))
                xt_t = xt_tiles[(ci, o)]
                for i in range(2):
                    m4 = 2 * half + i
                    nc.tensor.matmul(
                        pvd[(ci, half, i)][:],
                        xt_t[:, 128 * m4:128 * (m4 + 1)],
                        wv_s[:, o, :], **st
                    )
                if o == NO - 1:
                    for i in range(2):
                        nc.scalar.copy(
                            v_sb[:, 4 * ci + 2 * half + i, :],
                            pvd[(ci, half, i)][:],
                        )
            return f

        def a_stream(ci):
            return (
                [a_q_unit(ci, o) for o in range(NO)]
                + [a_v_unit(ci, o, 0) for o in range(NO)]
                + [a_v_unit(ci, o, 1) for o in range(NO)]
                + [a_k_unit(ci, o) for o in range(NO)]
            )

        def b_stream(ci):
            s0 = 512 * ci
            sb0 = 4 * ci
            nb = 4 * (ci + 1)
            units = []
            for h in range(HPC):
                st = {}

                def mk_pss(b, h=h, st=st):
                    c0 = 128 * (b - sb0) if b >= sb0 else 0
                    t = psB.tile([128, 512], F32, tag="ps",
                                 name=f"pss{ci}_{h}_{b}")
                    st[("pss", b)] = (t, c0)
                    nc.tensor.matmul(
                        t[:, c0:], kT[h][:, 128 * b:128 * (b + 1)],
                        qT[h][:, s0 + c0:s0 + 512], start=True, stop=True,
                    )

                def mk_post(b, h=h, st=st):
                    t, c0 = st[("pss", b)]
                    pt = ptp.tile([128, 512], F16, tag="pt",
                                  name=f"pt{ci}_{h}_{b}")
                    nc.scalar.activation(
                        pt[:, c0:], t[:, c0:],
                        mybir.ActivationFunctionType.Exp, scale=SCALE,
                    )
                    if b >= sb0:
                        nc.vector.tensor_mul(
                            pt[:, c0:c0 + 128], pt[:, c0:c0 + 128], tri[:]
                        )
                    nc.tensor.matmul(
                        st["po"][:, c0:], v_sb[:, b, 128 * h:128 * (h + 1)],
                        pt[:, c0:], start=(b == 0), stop=(b == nb - 1),
                    )
                    nc.tensor.matmul(
                        st["pd"][:, c0:], onc[:], pt[:, c0:],
                        start=(b == 0), stop=(b == nb - 1),
                    )

                def u0(h=h, st=st, mps=mk_pss):
                    st["po"] = psO.tile([128, 512], F32, tag="po",
                                        name=f"po{ci}_{h}")
                    st["pd"] = psD.tile([1, 512], F32, tag="pd",
                                        name=f"pd{ci}_{h}")
                    mps(0)
                    mps(1)

                def ub(b, h=h, st=st, mps=mk_pss, mpo=mk_post):
                    if b + 2 < nb:
                        mps(b + 2)
                    mpo(b)

                def ufin(h=h, st=st):
                    rec = recp.tile([1, 512], F32, tag="rec",
                                    name=f"rec{ci}_{h}")
                    with nc.allow_low_precision("softmax denom recip"):
                        nc.vector.reciprocal(rec[:], st["pd"][:])
                    bc = recp.tile([128, 512], F32, tag="bc",
                                   name=f"bc{ci}_{h}")
                    nc.gpsimd.partition_broadcast(bc[:], rec[:])
                    nc.vector.tensor_mul(
                        aT[h][:, s0:s0 + 512], st["po"][:], bc[:]
                    )

                units.append(u0)
                units.extend(
                    (lambda b=b, f=ub: f(b)) for b in range(nb)
                )
                units.append(ufin)
            return units

        def c_stream(ci):
            otd = {}
            units = []

            def unit(m4, e):
                m = 4 * ci + m4
                if e == 0:
                    otd[m4] = outp.tile([128, D], F16, tag="ot",
                                        name=f"ot{ci}_{m4}")
                pf = psB.tile([128, 512], F32, tag="ps",
                              name=f"pf{ci}_{m4}_{e}")
                for h2 in range(HPC):
                    nc.tensor.matmul(
                        pf[:], aT[h2][:, 128 * m:128 * (m + 1)],
                        wo_s[:, h2, 512 * e:512 * (e + 1)],
                        start=(h2 == 0), stop=(h2 == HPC - 1),
                    )
                if e % 2 == 0:
                    nc.vector.tensor_copy(otd[m4][:, 512 * e:512 * (e + 1)],
                                          pf[:])
                else:
                    nc.scalar.copy(otd[m4][:, 512 * e:512 * (e + 1)], pf[:])
                if e == 3:
                    nc.sync.dma_start(out_d[128 * m:128 * (m + 1), :],
                                      otd[m4][:])

            for m4 in range(4):
                for e in range(4):
                    units.append(lambda m4=m4, e=e: unit(m4, e))
            return units

        def run(units):
            for u in units:
                u()

        def interleave(streams):
            items = []
            for si, s in enumerate(streams):
                n = len(s)
                for j, u in enumerate(s):
                    items.append(((j + 0.5) / n, si, j, u))
            items.sort(key=lambda t: (t[0], t[1], t[2]))
            for it in items:
                it[3]()

        # ---- software-pipelined emission ----
        def one_rep():
            xt_dma(1)
            interleave([b_stream(0), a_stream(1)])
            xt_dma(2)
            interleave([b_stream(1), a_stream(2), c_stream(0)])
            xt_dma(3)
            interleave([b_stream(2), a_stream(3), c_stream(1)])
            interleave([b_stream(3), c_stream(2)])
            # Tail: chunk-3 out-projection interleaved with the next rep's
            # chunk-0 projections (loop-carried; the prologue feeds rep 0).
            xt_dma(0)
            interleave([c_stream(3), a_stream(0)])

        # Prologue: chunk-0 projections for the first rep (outside the loop).
        xt_dma(0)
        run(a_stream(0))

        if reps == 1:
            one_rep()
        else:
            # Two reps per For_i iteration halves the per-iteration
            # all-engine-barrier + semaphore-reset cost (~24us measured).
            half, rem = divmod(reps, 2)
            hint = tuple(
                getattr(mybir.EngineType, e)
                for e in ("PE", "DVE", "Activation", "SP", "Pool")
            )
            with tc.For_i(0, half, 1, hint_engines=hint):
                one_rep()
                one_rep()
            for _ in range(rem):
                one_rep()

    nc.compile()
    return nc


def _host_inputs(x, Wq, Wk, Wv, Wo):
    x = np.asarray(x, dtype=np.float32).reshape(S, D)
    xt = np.ascontiguousarray(x.T).astype(np.float16)

    half = DH // 2
    inv_freq = (
        ROT_FACTOR
        / (ROPE_BASE ** (np.arange(0, half, dtype=np.float32) * 2.0 / DH))
    ).astype(np.float32)
    sgn = np.where(np.arange(DH) < half, -1.0, 1.0).astype(np.float32)[:, None]
    fd = np.concatenate([inv_freq, inv_freq]).astype(np.float32)[:, None]
    u = np.arange(512, dtype=np.float32)[None, :]
    jj = (512.0 * np.arange(NJ, dtype=np.float32))[None, :]
    ropecu = np.cos(fd * u).astype(np.float32)
    ropesu = (sgn * np.sin(fd * u)).astype(np.float32)
    ropec512 = np.cos(fd * jj).astype(np.float32)
    ropes512 = (sgn * np.sin(fd * jj)).astype(np.float32)

    onescol = np.ones((128, 1), dtype=np.float16)

    Wq = np.asarray(Wq, dtype=np.float32)
    Wk = np.asarray(Wk, dtype=np.float32)
    Wv = np.asarray(Wv, dtype=np.float32)
    Wo = np.asarray(Wo, dtype=np.float32)

    in_maps = []
    for c in range(NCORES):
        sl = slice(CW * c, CW * (c + 1))
        in_maps.append(
            {
                "xt": xt,
                "wq": np.ascontiguousarray(Wq[:, sl]).astype(np.float16),
                "wk": np.ascontiguousarray(Wk[:, sl]).astype(np.float16),
                "wv": np.ascontiguousarray(Wv[:, sl]).astype(np.float16),
                "wo": np.ascontiguousarray(Wo[sl, :]).astype(np.float16),
                "ropecu": ropecu,
                "ropesu": ropesu,
                "ropec512": ropec512,
                "ropes512": ropes512,
                "onescol": onescol,
            }
        )
    return in_maps


_NC_CACHE = None


def kernel(x, Wq, Wk, Wv, Wo):
    global _NC_CACHE
    if _NC_CACHE is None:
        _NC_CACHE = _build_nc()
    in_maps = _host_inputs(x, Wq, Wk, Wv, Wo)
    res = run_bass_kernel_spmd(_NC_CACHE, in_maps, core_ids=list(range(NCORES)))
    out = np.zeros((S, D), dtype=np.float32)
    for r in res.results:
        out += r["out"].astype(np.float32)
    return out.reshape(1, S, D)



# revision 8
# speedup vs baseline: 1.0127x; 1.0127x over previous
"""PhiHarmonicAttention (B=1, S=2048, D=2048, H=16, Dh=128) on 8 Trainium2 cores.

Sharding: tensor-parallel over heads - 2 heads per core.
  - Wq/Wk/Wv column-sliced (256 cols per core), Wo row-sliced (256 rows).
  - Host sums the 8 partial outputs (TP row-parallel reduction).

v4: on top of v3 (fp16 operands, exact causal clipping, software-pipelined
A/B/C emission):
  - reps-per-For_i-iteration raised 2 -> 4 (quarter the all-engine-barrier +
    semaphore-reset cost per rep).
  - pd_ct: softmax-denominator ones-matmuls col-tiled 4x via tile_position
    (0, 32g) so 4 of them stream concurrently through disjoint PE column
    groups (~1/4 the PE cycles); the 4 partial-denominator rows are summed by
    one tiny [128,1]-stationary matmul per (head, chunk) after a DVE
    evacuation.  Chunk 0 keeps the single-row path (its only block per group
    is diagonal-clipped, which would leave stale PSUM in the clipped cols).
"""
import numpy as np
from contextlib import ExitStack, nullcontext

import concourse.bass as bass
import concourse.tile as tile
from concourse import bacc, mybir
from concourse.bass_utils import run_bass_kernel_spmd

S = 2048
D = 2048
H = 16
DH = 128
NCORES = 8
HPC = H // NCORES          # heads per core = 2
CW = HPC * DH              # weight col-slice per core = 256
NO = D // 128              # contraction chunks = 16
NJ = S // 512              # rope table chunks = 4
NB = S // 128              # 128-wide seq blocks = 16
SCALE = float(1.0 / np.sqrt(np.float32(DH)))

ROT_FACTOR = (1.0 + 5.0 ** 0.5) / 2.0 - 1.0
ROPE_BASE = 10000.0

F32 = mybir.dt.float32
F32R = mybir.dt.float32r
F16 = mybir.dt.float16


def _build_nc(reps=1, rpi=4, pd_ct=True, only=None, xt_loop_dma=True):
    nc = bacc.Bacc("TRN2", target_bir_lowering=False, debug=False, num_devices=NCORES)

    xt_d = nc.dram_tensor("xt", [D, S], F16, kind="ExternalInput").ap()
    wq_d = nc.dram_tensor("wq", [D, CW], F16, kind="ExternalInput").ap()
    wk_d = nc.dram_tensor("wk", [D, CW], F16, kind="ExternalInput").ap()
    wv_d = nc.dram_tensor("wv", [D, CW], F16, kind="ExternalInput").ap()
    wo_d = nc.dram_tensor("wo", [CW, D], F16, kind="ExternalInput").ap()
    rcu_d = nc.dram_tensor("ropecu", [DH, 512], F32, kind="ExternalInput").ap()
    rsu_d = nc.dram_tensor("ropesu", [DH, 512], F32, kind="ExternalInput").ap()
    rc512_d = nc.dram_tensor("ropec512", [DH, NJ], F32, kind="ExternalInput").ap()
    rs512_d = nc.dram_tensor("ropes512", [DH, NJ], F32, kind="ExternalInput").ap()
    onc_d = nc.dram_tensor("onescol", [128, 1], F16, kind="ExternalInput").ap()
    sel4_d = nc.dram_tensor("sel4col", [128, 1], F16, kind="ExternalInput").ap()
    out_d = nc.dram_tensor("out", [S, D], F16, kind="ExternalOutput").ap()

    with ExitStack() as ctx:
        tc = ctx.enter_context(tile.TileContext(nc))
        consts = ctx.enter_context(tc.tile_pool(name="consts", bufs=1))
        persist = ctx.enter_context(tc.tile_pool(name="persist", bufs=1))
        xw = ctx.enter_context(tc.tile_pool(name="xw", bufs=28 if only is None else 48))
        ptp = ctx.enter_context(tc.tile_pool(name="ptp", bufs=8))
        work = ctx.enter_context(tc.tile_pool(name="work", bufs=3))
        recp = ctx.enter_context(tc.tile_pool(name="recp", bufs=3))
        outp = ctx.enter_context(tc.tile_pool(name="outp", bufs=4))
        psQ = ctx.enter_context(tc.tile_pool(name="psQ", bufs=3, space="PSUM"))
        psB = ctx.enter_context(tc.tile_pool(name="psB", bufs=3, space="PSUM"))
        psO = ctx.enter_context(tc.tile_pool(name="psO", bufs=1, space="PSUM"))
        psD = ctx.enter_context(tc.tile_pool(name="psD", bufs=1, space="PSUM"))

        # ---- constants ----
        wq_s = consts.tile([128, NO, CW], F16, tag="wq")
        wk_s = consts.tile([128, NO, CW], F16, tag="wk")
        wv_s = consts.tile([128, NO, CW], F16, tag="wv")
        wo_s = consts.tile([128, HPC, D], F16, tag="wo")
        rc = consts.tile([DH, S], F16, tag="rc")
        rs = consts.tile([DH, S], F16, tag="rs")
        tri = consts.tile([128, 128], F16, tag="tri")
        onc = consts.tile([128, 1], F16, tag="onc")
        sel4 = consts.tile([128, 1], F16, tag="sel4")
        rcu = consts.tile([DH, 512], F32, tag="rcu")
        rsu = consts.tile([DH, 512], F32, tag="rsu")
        rc512 = consts.tile([DH, NJ], F32, tag="rc512")
        rs512 = consts.tile([DH, NJ], F32, tag="rs512")

        nc.sync.dma_start(rcu[:], rcu_d)
        nc.sync.dma_start(rsu[:], rsu_d)
        nc.sync.dma_start(rc512[:], rc512_d)
        nc.sync.dma_start(rs512[:], rs512_d)
        nc.sync.dma_start(wq_s[:], wq_d.rearrange("(o p) n -> p o n", p=128))
        nc.sync.dma_start(wv_s[:], wv_d.rearrange("(o p) n -> p o n", p=128))
        nc.sync.dma_start(wk_s[:], wk_d.rearrange("(o p) n -> p o n", p=128))
        nc.sync.dma_start(onc[:], onc_d)
        nc.sync.dma_start(sel4[:], sel4_d)
        nc.sync.dma_start(wo_s[:], wo_d.rearrange("(h p) n -> p h n", p=128))

        # rc/rs = full [128, 2048] rope tables via angle addition from the
        # 512-wide unit tables.
        for j in range(NJ):
            sl = slice(512 * j, 512 * (j + 1))
            tm = work.tile([128, 512], F32, tag="tm", name=f"tm{j}")
            nc.vector.tensor_scalar_mul(tm[:], rsu[:], rs512[:, j:j + 1])
            nc.vector.scalar_tensor_tensor(
                rc[:, sl], rcu[:], rc512[:, j:j + 1], tm[:],
                mybir.AluOpType.mult, mybir.AluOpType.subtract,
            )
            tm2 = work.tile([128, 512], F32, tag="tm2", name=f"tm2{j}")
            nc.vector.tensor_scalar_mul(tm2[:], rcu[:], rs512[:, j:j + 1])
            nc.vector.scalar_tensor_tensor(
                rs[:, sl], rsu[:], rc512[:, j:j + 1], tm2[:],
                mybir.AluOpType.mult, mybir.AluOpType.add,
            )
        # tri[p, c] = 1 if c >= p else 0  (within-block causal triangle)
        iot = work.tile([128, 512], F32, tag="tm", name="iot")
        nc.gpsimd.iota(
            iot[:, :128], pattern=[[1, 128]], base=0, channel_multiplier=-1,
            allow_small_or_imprecise_dtypes=True,
        )
        nc.vector.tensor_scalar(
            tri[:], iot[:, :128], 0.0, None, mybir.AluOpType.is_ge,
        )

        # ---- persistent per-head tensors ----
        # Single persistent denominator PSUM bank shared by all (head, chunk)
        # streams.  Rows {0,32,64,96} carry the 4 col-tiled partial
        # denominators (chunk 0 uses only row 0); all other rows are zeroed
        # once here and never written, so the fp16 evacuation below reads
        # well-defined zeros.
        pdt = psD.tile([128, 512], F32, tag="pd", name="pdt")
        if pd_ct:
            nc.vector.memset(pdt[:], 0.0)
        qT = [persist.tile([DH, S], F16, tag=f"qT{h}", name=f"qT{h}")
              for h in range(HPC)]
        kT = [persist.tile([DH, S], F16, tag=f"kT{h}", name=f"kT{h}")
              for h in range(HPC)]
        v_sb = persist.tile([128, NB, CW], F16, tag="v")
        aT = [persist.tile([DH, S], F16, tag=f"aT{h}", name=f"aT{h}")
              for h in range(HPC)]

        xt_tiles = {}
        pqd, pkd, pvd = {}, {}, {}

        def xt_dma(ci):
            s0 = 512 * ci
            for o2 in range(NO // 2):
                t = xw.tile([128, 2, 512], F16, tag="xt", name=f"xt{ci}_{o2}")
                nc.sync.dma_start(
                    t[:],
                    xt_d[256 * o2:256 * (o2 + 1), s0:s0 + 512].rearrange(
                        "(a p) n -> p a n", p=128
                    ),
                )
                xt_tiles[(ci, 2 * o2)] = t[:, 0, :]
                xt_tiles[(ci, 2 * o2 + 1)] = t[:, 1, :]

        def rope_emit(psrc, dstT, s0):
            # dstT = raw*cos + halfswap(raw)*sin  (sign folded into rs)
            sl = slice(s0, s0 + 512)
            raw = work.tile([128, 512], F16, tag="raw")
            nc.scalar.copy(raw[:], psrc[:])          # frees the PSUM bank
            rsw = work.tile([128, 512], F16, tag="rsw")
            nc.gpsimd.tensor_copy(rsw[0:64, :], raw[64:128, :])
            nc.gpsimd.tensor_copy(rsw[64:128, :], raw[0:64, :])
            t1 = work.tile([128, 512], F16, tag="t1")
            nc.vector.tensor_mul(t1[:], raw[:], rc[:, sl])
            nc.vector.tensor_mul(rsw[:], rsw[:], rs[:, sl])
            nc.vector.tensor_add(dstT[:, sl], t1[:], rsw[:])

        def a_q_unit(ci, o):
            def f():
                if o == 0:
                    for h in range(HPC):
                        pqd[(ci, h)] = psQ.tile(
                            [128, 512], F32, tag="ps", name=f"pq{ci}_{h}"
                        )
                st = dict(start=(o == 0), stop=(o == NO - 1))
                xt_t = xt_tiles[(ci, o)]
                for h in range(HPC):
                    nc.tensor.matmul(
                        pqd[(ci, h)][:], wq_s[:, o, 128 * h:128 * (h + 1)],
                        xt_t, **st
                    )
                if o == NO - 1:
                    for h in range(HPC):
                        rope_emit(pqd[(ci, h)], qT[h], 512 * ci)
            return f

        def a_k_unit(ci, o):
            def f():
                if o == 0:
                    for h in range(HPC):
                        pkd[(ci, h)] = psQ.tile(
                            [128, 512], F32, tag="ps", name=f"pk{ci}_{h}"
                        )
                st = dict(start=(o == 0), stop=(o == NO - 1))
                xt_t = xt_tiles[(ci, o)]
                for h in range(HPC):
                    nc.tensor.matmul(
                        pkd[(ci, h)][:], wk_s[:, o, 128 * h:128 * (h + 1)],
                        xt_t, **st
                    )
                if o == NO - 1:
                    for h in range(HPC):
                        rope_emit(pkd[(ci, h)], kT[h], 512 * ci)
            return f

        def a_v_unit(ci, o, half):
            # v projection in two sequential half-passes (m4 in {0,1} then
            # {2,3}) so only 2 of the 3 psQ banks are held at a time.
            def f():
                if o == 0:
                    for i in range(2):
                        pvd[(ci, half, i)] = psQ.tile(
                            [128, 256], F32, tag="ps",
                            name=f"pv{ci}_{half}_{i}"
                        )
                st = dict(start=(o == 0), stop=(o == NO - 1))
                xt_t = xt_tiles[(ci, o)]
                for i in range(2):
                    m4 = 2 * half + i
                    nc.tensor.matmul(
                        pvd[(ci, half, i)][:],
                        xt_t[:, 128 * m4:128 * (m4 + 1)],
                        wv_s[:, o, :], **st
                    )
                if o == NO - 1:
                    for i in range(2):
                        nc.scalar.copy(
                            v_sb[:, 4 * ci + 2 * half + i, :],
                            pvd[(ci, half, i)][:],
                        )
            return f

        def a_stream(ci):
            return (
                [a_q_unit(ci, o) for o in range(NO)]
                + [a_v_unit(ci, o, 0) for o in range(NO)]
                + [a_v_unit(ci, o, 1) for o in range(NO)]
                + [a_k_unit(ci, o) for o in range(NO)]
            )

        def b_stream(ci):
            s0 = 512 * ci
            sb0 = 4 * ci
            nb = 4 * (ci + 1)
            ct = pd_ct and ci > 0   # chunk 0: single diagonal block per
            #                         group would leave stale PSUM cols
            units = []
            for h in range(HPC):
                st = {}

                def mk_pss(b, h=h, st=st):
                    c0 = 128 * (b - sb0) if b >= sb0 else 0
                    t = psB.tile([128, 512], F32, tag="ps",
                                 name=f"pss{ci}_{h}_{b}")
                    st[("pss", b)] = (t, c0)
                    nc.tensor.matmul(
                        t[:, c0:], kT[h][:, 128 * b:128 * (b + 1)],
                        qT[h][:, s0 + c0:s0 + 512], start=True, stop=True,
                    )

                def mk_post(b, h=h, st=st, ct=ct, nb=nb):
                    t, c0 = st[("pss", b)]
                    pt = ptp.tile([128, 512], F16, tag="pt",
                                  name=f"pt{ci}_{h}_{b}")
                    nc.scalar.activation(
                        pt[:, c0:], t[:, c0:],
                        mybir.ActivationFunctionType.Exp, scale=SCALE,
                    )
                    if b >= sb0:
                        nc.vector.tensor_mul(
                            pt[:, c0:c0 + 128], pt[:, c0:c0 + 128], tri[:]
                        )
                    nc.tensor.matmul(
                        st["po"][:, c0:], v_sb[:, b, 128 * h:128 * (h + 1)],
                        pt[:, c0:], start=(b == 0), stop=(b == nb - 1),
                    )
                    if not ct:
                        nc.tensor.matmul(
                            st["pd"][0:1, c0:], onc[:], pt[:, c0:],
                            start=(b == 0), stop=(b == nb - 1),
                        )
                    else:
                        st[("pt", b)] = (pt, c0)
                        if b % 4 == 3:
                            # col-tiled concurrent denominator matmuls: group
                            # g accumulates blocks b%4==g into psD partition
                            # row 32g via PE column-group g.  start=True
                            # clears has_written only for the written
                            # partitions, so each group starts its own row.
                            for g in range(4):
                                bb = b - 3 + g
                                ptg, c0g = st.pop(("pt", bb))
                                nc.tensor.matmul(
                                    st["pd"][32 * g:32 * g + 1, c0g:],
                                    onc[:], ptg[:, c0g:],
                                    start=(bb < 4), stop=(bb >= nb - 4),
                                    tile_position=(0, 32 * g),
                                    skip_group_check=True,
                                )

                def u0(h=h, st=st, mps=mk_pss):
                    st["po"] = psO.tile([128, 512], F32, tag="po",
                                        name=f"po{ci}_{h}")
                    st["pd"] = pdt
                    mps(0)
                    mps(1)

                def ub(b, h=h, st=st, mps=mk_pss, mpo=mk_post, nb=nb):
                    if b + 2 < nb:
                        mps(b + 2)
                    mpo(b)

                def ufin(h=h, st=st, ct=ct, ci=ci):
                    rec = recp.tile([1, 512], F32, tag="rec",
                                    name=f"rec{ci}_{h}")
                    if ct:
                        sb4 = recp.tile([128, 512], F16, tag="sb4",
                                        name=f"sb4{ci}_{h}")
                        nc.vector.tensor_copy(sb4[:], st["pd"][:])
                        pd2 = psB.tile([1, 512], F32, tag="ps",
                                       name=f"pd2{ci}_{h}")
                        nc.tensor.matmul(pd2[:], sel4[:], sb4[:],
                                         start=True, stop=True)
                        den = pd2
                    else:
                        den = st["pd"][0:1, :]
                    with nc.allow_low_precision("softmax denom recip"):
                        nc.vector.reciprocal(rec[:], den[:])
                    bc = recp.tile([128, 512], F32, tag="bc",
                                   name=f"bc{ci}_{h}")
                    nc.gpsimd.partition_broadcast(bc[:], rec[:])
                    nc.vector.tensor_mul(
                        aT[h][:, 512 * ci:512 * ci + 512], st["po"][:], bc[:]
                    )

                units.append(u0)
                units.extend(
                    (lambda b=b, f=ub: f(b)) for b in range(nb)
                )
                units.append(ufin)
            return units

        def c_stream(ci):
            otd = {}
            units = []

            def unit(m4, e):
                m = 4 * ci + m4
                if e == 0:
                    otd[m4] = outp.tile([128, D], F16, tag="ot",
                                        name=f"ot{ci}_{m4}")
                pf = psB.tile([128, 512], F32, tag="ps",
                              name=f"pf{ci}_{m4}_{e}")
                for h2 in range(HPC):
                    nc.tensor.matmul(
                        pf[:], aT[h2][:, 128 * m:128 * (m + 1)],
                        wo_s[:, h2, 512 * e:512 * (e + 1)],
                        start=(h2 == 0), stop=(h2 == HPC - 1),
                    )
                if e % 2 == 0:
                    nc.vector.tensor_copy(otd[m4][:, 512 * e:512 * (e + 1)],
                                          pf[:])
                else:
                    nc.scalar.copy(otd[m4][:, 512 * e:512 * (e + 1)], pf[:])
                if e == 3:
                    nc.sync.dma_start(out_d[128 * m:128 * (m + 1), :],
                                      otd[m4][:])

            for m4 in range(4):
                for e in range(4):
                    units.append(lambda m4=m4, e=e: unit(m4, e))
            return units

        def run(units):
            for u in units:
                u()

        def interleave(streams):
            items = []
            for si, s in enumerate(streams):
                n = len(s)
                for j, u in enumerate(s):
                    items.append(((j + 0.5) / n, si, j, u))
            items.sort(key=lambda t: (t[0], t[1], t[2]))
            for it in items:
                it[3]()

        # ---- software-pipelined emission ----
        def one_rep():
            xt_dma(1)
            interleave([b_stream(0), a_stream(1)])
            xt_dma(2)
            interleave([b_stream(1), a_stream(2), c_stream(0)])
            xt_dma(3)
            interleave([b_stream(2), a_stream(3), c_stream(1)])
            interleave([b_stream(3), c_stream(2)])
            # Tail: chunk-3 out-projection interleaved with the next rep's
            # chunk-0 projections (loop-carried; the prologue feeds rep 0).
            xt_dma(0)
            interleave([c_stream(3), a_stream(0)])

        def one_rep_a_only():
            # Probe body: projections only (bench diagnostics, never used
            # for real output).  Chunk ci+1's DMA overlaps chunk ci's
            # compute; the wrap-around re-DMA of chunk 0 overlaps chunk 3.
            for ci in range(NJ):
                if xt_loop_dma:
                    xt_dma((ci + 1) % NJ)
                run(a_stream(ci))

        body = one_rep if only is None else one_rep_a_only

        # Prologue: chunk-0 projections for the first rep (outside the loop).
        if only is None:
            xt_dma(0)
            run(a_stream(0))
        else:
            if xt_loop_dma:
                xt_dma(0)
            else:
                for ci in range(NJ):
                    xt_dma(ci)

        if reps == 1:
            body()
        else:
            n_iter, rem = divmod(reps, rpi)
            hint = tuple(
                getattr(mybir.EngineType, e)
                for e in ("PE", "DVE", "Activation", "SP", "Pool")
            )
            with tc.For_i(0, n_iter, 1, hint_engines=hint):
                for _ in range(rpi):
                    body()
            for _ in range(rem):
                body()

    nc.compile()
    return nc


def _host_inputs(x, Wq, Wk, Wv, Wo):
    x = np.asarray(x, dtype=np.float32).reshape(S, D)
    xt = np.ascontiguousarray(x.T).astype(np.float16)

    half = DH // 2
    inv_freq = (
        ROT_FACTOR
        / (ROPE_BASE ** (np.arange(0, half, dtype=np.float32) * 2.0 / DH))
    ).astype(np.float32)
    sgn = np.where(np.arange(DH) < half, -1.0, 1.0).astype(np.float32)[:, None]
    fd = np.concatenate([inv_freq, inv_freq]).astype(np.float32)[:, None]
    u = np.arange(512, dtype=np.float32)[None, :]
    jj = (512.0 * np.arange(NJ, dtype=np.float32))[None, :]
    ropecu = np.cos(fd * u).astype(np.float32)
    ropesu = (sgn * np.sin(fd * u)).astype(np.float32)
    ropec512 = np.cos(fd * jj).astype(np.float32)
    ropes512 = (sgn * np.sin(fd * jj)).astype(np.float32)

    onescol = np.ones((128, 1), dtype=np.float16)
    sel4col = np.zeros((128, 1), dtype=np.float16)
    sel4col[0::32] = 1.0

    Wq = np.asarray(Wq, dtype=np.float32)
    Wk = np.asarray(Wk, dtype=np.float32)
    Wv = np.asarray(Wv, dtype=np.float32)
    Wo = np.asarray(Wo, dtype=np.float32)

    in_maps = []
    for c in range(NCORES):
        sl = slice(CW * c, CW * (c + 1))
        in_maps.append(
            {
                "xt": xt,
                "wq": np.ascontiguousarray(Wq[:, sl]).astype(np.float16),
                "wk": np.ascontiguousarray(Wk[:, sl]).astype(np.float16),
                "wv": np.ascontiguousarray(Wv[:, sl]).astype(np.float16),
                "wo": np.ascontiguousarray(Wo[sl, :]).astype(np.float16),
                "ropecu": ropecu,
                "ropesu": ropesu,
                "ropec512": ropec512,
                "ropes512": ropes512,
                "onescol": onescol,
                "sel4col": sel4col,
            }
        )
    return in_maps


_NC_CACHE = None


def kernel(x, Wq, Wk, Wv, Wo):
    global _NC_CACHE
    if _NC_CACHE is None:
        _NC_CACHE = _build_nc()
    in_maps = _host_inputs(x, Wq, Wk, Wv, Wo)
    res = run_bass_kernel_spmd(_NC_CACHE, in_maps, core_ids=list(range(NCORES)))
    out = np.zeros((S, D), dtype=np.float32)
    for r in res.results:
        out += r["out"].astype(np.float32)
    return out.reshape(1, S, D)
